# revision 1
# baseline (speedup 1.0000x reference)
"""Self-contained Trainium2 kernel for nn_B3SplineUWT (3-level B3-spline
undecimated wavelet transform), data-parallel over 8 NeuronCores.

kernel(x: [8,1024,1024] f32) -> [8,4,1024,1024] f32  (w1,w2,w3,c3)

Per core: one image. fp32r data path (~1e-4 rel).
  - H-conv (partition dim): PE banded matmuls (fp32r, exact given inputs)
  - W-conv: chunks 0..NPE-1 via PE transpose sandwich (fp32r transposes
    + same banded matmuls over w); chunks NPE..7 via DVE shifted-AP
    passes (one pass on GPSIMD).
  - ACT: PSUM evacuations + margins. DVE: normalize. subs split DVE/POOL.
  - Coalesced 2MB DMAs.
"""
import numpy as np

import concourse.bacc as bacc
import concourse.bass as bass
import concourse.mybir as mybir
import concourse.tile as tile
from concourse.bass_utils import run_bass_kernel_spmd

F32 = mybir.dt.float32
F32R = mybir.dt.float32r
ADD = mybir.AluOpType.add
MULT = mybir.AluOpType.mult

B = 8
H = 1024
W = 1024
P = 128
NCH = H // P
LEVELS = 3
DILS = (1, 2, 4)
MARG = 8
WE = W + 2 * MARG

NPE = 4            # h-chunks routed via PE transpose sandwich
HPE = NPE * P

TAPS = {0: 3.0 / 8, 1: 1.0 / 4, 2: 1.0 / 16}


def _reflect(i, n):
    if i < 0:
        return -i
    if i >= n:
        return 2 * (n - 1) - i
    return i


def _build_h_bands():
    out = []
    for j, d in enumerate(DILS):
        full = np.zeros((H, H), np.float32)
        for r in range(H):
            for o in (-2 * d, -d, 0, d, 2 * d):
                full[_reflect(r + o, H), r] += TAPS[abs(o) // d]
        blocks = {}
        for co in range(NCH):
            for ci in range(NCH):
                blk = full[ci * P:(ci + 1) * P, co * P:(co + 1) * P]
                if np.any(blk != 0):
                    blocks[(co, ci)] = np.ascontiguousarray(blk)
        out.append(blocks)
    return out


def _pack_consts(h_bands):
    mats, seen, index = [], {}, []
    for blocks in h_bands:
        idx = {}
        for key in sorted(blocks):
            b = blocks[key]
            hsh = b.tobytes()
            if hsh not in seen:
                seen[hsh] = len(mats) * P
                mats.append(b)
            idx[key] = seen[hsh]
        index.append(idx)
    ident_off = len(mats) * P
    mats.append(np.eye(P, dtype=np.float32))
    packed = np.ascontiguousarray(
        np.concatenate(mats, axis=1).astype(np.float32))
    return packed, index, ident_off


def _build_program(npe=3, hbufs=2, tbufs=2, reps=1, use_pool=True, rdt=None):
    R = rdt if rdt is not None else F32R

    def asf(ap):
        return ap.bitcast(F32) if R == F32R else ap

    NPE = npe if npe is not None else globals()['NPE']
    HPE = NPE * P if NPE > 0 else 512
    h_bands = _build_h_bands()
    consts_np, cindex, ident_off = _pack_consts(h_bands)
    ncols_const = consts_np.shape[1]

    nc = bacc.Bacc("TRN2", target_bir_lowering=False, debug=False)
    x_d = nc.dram_tensor("x", [H, W], F32, kind="ExternalInput")
    c_d = nc.dram_tensor("consts", [P, ncols_const], F32, kind="ExternalInput")
    out_d = nc.dram_tensor("out", [LEVELS + 1, H, W], F32,
                           kind="ExternalOutput")

    with tile.TileContext(nc) as tc:
        with tc.tile_pool(name="sb", bufs=1) as sb, \
             tc.tile_pool(name="wst", bufs=2) as wstage, \
             tc.tile_pool(name="ytmp", bufs=2) as ypool, \
             tc.tile_pool(name="yrp", bufs=max(NPE, 1)) as yrpool, \
             tc.tile_pool(name="wtmp", bufs=2) as wtmp, \
             tc.tile_pool(name="ps", bufs=4, space="PSUM") as ps:

            cr = sb.tile([P, ncols_const], R, tag="cr", name="cr")
            cf = wstage.tile([P, 4, W], F32, tag="wstage", name="cf")
            cf_flat = bass.AP(cf.tensor, 0, [[4 * W, P], [1, ncols_const]])
            nc.sync.dma_start(cf_flat, c_d[:])
            nc.vector.tensor_copy(cr[:], cf_flat)
            ident = cr[:, ident_off:ident_off + P]

            xr = sb.tile([P, NCH, W], R, tag="xr", name="xr")

            cnr = [sb.tile([P, NCH, W], R, tag=f"cnr{i}", name=f"cnr{i}")
                   for i in range(2)]
            if NPE > 0:
                tT = sb.tile([P, NCH, HPE], R, tag="tT", name="tT")
                ct = sb.tile([P, NCH, HPE], R, tag="ct", name="ct")

            for rep in range(reps):
              # ---- load x: 2 x 2MB DMAs, round to fp32r ----
              for hv in range(2):
                xs = wstage.tile([P, 4, W], F32, tag="wstage", name="xs")
                nc.sync.dma_start(
                    xs[:], bass.AP(x_d, hv * 4 * P * W,
                                   [[W, P], [P * W, 4], [1, W]]))
                nc.vector.tensor_copy(xr[:, hv * 4:(hv + 1) * 4, :], xs[:])
              for j in range(LEVELS):
                d = DILS[j]
                inr = xr if j == 0 else cnr[(j - 1) % 2]
                cur = cnr[j % 2]

                # ---------- H-conv on PE, all chunks ----------
                yrs = {}
                yxs = {}
                for co in range(NCH):
                    pe_route = co < NPE
                    pairs = sorted((key, off)
                                   for key, off in cindex[j].items()
                                   if key[0] == co)
                    if pe_route:
                        yr = yrpool.tile([P, W], R, tag="yr", name="yr")
                        yrs[co] = yr
                    else:
                        yx = ypool.tile([P, WE], F32, tag="yext", name="yx")
                        yxs[co] = yx
                    pt = ps.tile([P, W], F32, tag="psum", name="pt",
                                 bufs=4)
                    for half in range(2):
                        for i, ((_, ci), off) in enumerate(pairs):
                            nc.tensor.matmul(
                                pt[:, half * 512:(half + 1) * 512],
                                cr[:, off:off + P],
                                inr[:, ci, half * 512:(half + 1) * 512],
                                start=(i == 0),
                                stop=(i == len(pairs) - 1))
                    if pe_route:
                        nc.scalar.copy(yr[:], pt[:])
                    else:
                        nc.scalar.copy(yx[:, MARG:MARG + W], pt[:])
                    if not pe_route:
                        yx = yxs[co]
                        meng = nc.gpsimd if use_pool else nc.scalar
                        meng.tensor_copy(
                            bass.AP(yx.tensor, 0, [[WE, P], [1, MARG]]),
                            bass.AP(yx.tensor, 2 * MARG,
                                    [[WE, P], [-1, MARG]])) if use_pool else meng.copy(
                            bass.AP(yx.tensor, 0, [[WE, P], [1, MARG]]),
                            bass.AP(yx.tensor, 2 * MARG,
                                    [[WE, P], [-1, MARG]]))
                        meng.tensor_copy(
                            bass.AP(yx.tensor, MARG + W, [[WE, P], [1, MARG]]),
                            bass.AP(yx.tensor, MARG + W - 2,
                                    [[WE, P], [-1, MARG]])) if use_pool else meng.copy(
                            bass.AP(yx.tensor, MARG + W, [[WE, P], [1, MARG]]),
                            bass.AP(yx.tensor, MARG + W - 2,
                                    [[WE, P], [-1, MARG]]))

                # ---------- W-conv PE route: interleave T and Wband ----------
                def emit_T(q):
                    tpq = ps.tile([P, HPE], R, tag="psum", name="tpq",
                                  bufs=4)
                    for co in range(NPE):
                        nc.tensor.transpose(
                            tpq[:, co * P:(co + 1) * P],
                            yrs[co][:, q * P:(q + 1) * P],
                            ident)
                    nc.scalar.copy(tT[:, q, :], tpq[:])

                def emit_W(qo):
                    pairs = sorted((key, off)
                                   for key, off in cindex[j].items()
                                   if key[0] == qo)
                    pw = ps.tile([P, HPE], F32, tag="psum", name="pw",
                                 bufs=4)
                    for i, ((_, qi), off) in enumerate(pairs):
                        nc.tensor.matmul(
                            pw[:],
                            cr[:, off:off + P],
                            tT[:, qi, :],
                            start=(i == 0),
                            stop=(i == len(pairs) - 1))
                    nc.scalar.copy(ct[:, qo, :], pw[:])

                if NPE > 0:
                    emit_T(0)
                    emit_T(1)
                    for q in range(2, NCH):
                        emit_T(q)
                        emit_W(q - 1)
                    emit_W(0)
                    emit_W(NCH - 1)
                    for co in range(NPE):
                        tb = ps.tile([P, W], R, tag="psum", name="tb",
                                     bufs=4)
                        for q in range(NCH):
                            nc.tensor.transpose(
                                tb[:, q * P:(q + 1) * P],
                                ct[:, q, co * P:(co + 1) * P],
                                ident)
                        nc.scalar.copy(cur[:, co, :], tb[:])

                # ---------- W-conv DVE route (chunks NPE..7) ----------
                for co in range(NPE, NCH):
                    yx = yxs[co]

                    def ysh(o, yx=yx):
                        return bass.AP(yx.tensor, MARG + o, [[WE, P], [1, W]])
                    t1 = wtmp.tile([P, W], F32, tag="t1", name="t1", bufs=1)
                    t2 = wtmp.tile([P, W], F32, tag="t2", name="t2", bufs=2)
                    (nc.gpsimd if use_pool else nc.vector).tensor_add(
                        t2[:], ysh(-2 * d), ysh(2 * d))
                    nc.vector.tensor_add(t1[:], ysh(-d), ysh(d))
                    nc.vector.scalar_tensor_tensor(
                        t1[:], t1[:], 4.0, t2[:], op0=MULT, op1=ADD)
                    nc.vector.scalar_tensor_tensor(
                        t2[:], ysh(0), 6.0, t1[:], op0=MULT, op1=ADD)
                    # normalize + round: cn = c16/16 [DVE tensor_scalar 2x]
                    nc.vector.tensor_scalar_mul(cur[:, co, :], t2[:],
                                                1.0 / 16)

                # ---------- outputs: subs into half-plane staging ----------
                prev = xr if j == 0 else cnr[(j - 1) % 2]
                for hv in range(2):
                    wst = wstage.tile([P, 4, W], F32, tag="wstage",
                                      name="wst")
                    for ci_, co in enumerate(range(hv * 4, hv * 4 + 4)):
                        eng = nc.vector if (co % 2 == 0 or not use_pool) else nc.gpsimd
                        eng.tensor_sub(
                            wst[:, ci_, :], asf(prev[:, co, :]),
                            asf(cur[:, co, :]))
                    nc.sync.dma_start(
                        bass.AP(out_d, j * H * W + hv * 4 * P * W,
                                [[W, P], [P * W, 4], [1, W]]),
                        wst[:])
                if j == LEVELS - 1:
                    c3eng = nc.sync if R == F32R else nc.gpsimd
                    for hv in range(2):
                        c3eng.dma_start(
                            bass.AP(out_d, 3 * H * W + hv * 4 * P * W,
                                    [[W, P], [P * W, 4], [1, W]]),
                            asf(cur[:, hv * 4:(hv + 1) * 4, :]))

    nc.compile()
    return nc, consts_np


_CACHE = {}


def _get_program():
    if "prog" not in _CACHE:
        _CACHE["prog"] = _build_program()
    return _CACHE["prog"]


def kernel(x, _trace=False, _trace_kwargs=None):
    """x: [8, 1024, 1024] float32 -> [8, 4, 1024, 1024] float32."""
    x = np.asarray(x)
    assert x.shape == (B, H, W) and x.dtype == np.float32
    nc, consts_np = _get_program()
    in_maps = [{"x": np.ascontiguousarray(x[b]), "consts": consts_np}
               for b in range(B)]
    kw = {}
    if _trace:
        kw = dict(trace=True, **(_trace_kwargs or {}))
    res = run_bass_kernel_spmd(nc, in_maps, core_ids=list(range(B)), **kw)
    out = np.stack([r["out"] for r in res.results], axis=0)
    if _trace:
        return out, res
    return out



# revision 19
# speedup vs baseline: 1.6684x; 1.6684x over previous
"""Self-contained Trainium2 kernel for nn_B3SplineUWT (3-level B3-spline
undecimated wavelet transform), data-parallel over 8 NeuronCores.

kernel(x: [8,1024,1024] f32) -> [8,4,1024,1024] f32  (w1,w2,w3,c3)

Per core: one image, bf16 datapath (tolerance 2e-2; bf16 lands ~2e-3).
  - H-conv (partition dim): PE banded matmuls; taps pre-scaled by 1/16
    (exact in bf16) so the W-conv uses integer taps {1,2,1}x{1,2,1} /
    {1,4,6} with no normalization pass.
  - W-conv (free dim), per 512-half routable:
      route PE : 5 accumulating shift-matmuls (integer-scaled identity
                 stationaries, shifted moving APs)
      route DVE: factorized [1,2,1] twice = 4 adds + 2 scale-by-2
                 (TS ops optionally on ACT, first add optionally on Pool)
  - Subtracts w_j = c_{j-1} - c_j fused per 2-chunk pair on DVE.
  - All HBM I/O via SWDGE cast-DMAs (f32<->bf16 in the DMA), quarter/
    pair granularity for pipelining.
"""
import numpy as np

import concourse.bacc as bacc
import concourse.bass as bass
import concourse.mybir as mybir
import concourse.tile as tile
from concourse.bass_utils import run_bass_kernel_spmd

F32 = mybir.dt.float32
BF16 = mybir.dt.bfloat16
ADD = mybir.AluOpType.add
MULT = mybir.AluOpType.mult

B = 8
H = 1024
W = 1024
P = 128
NCH = H // P
LEVELS = 3
DILS = (1, 2, 4)
MARG = 8
WE = W + 2 * MARG

# H-conv taps with the 1/16 W-normalization folded in (exact in bf16)
HTAPS = {0: 3.0 / 128, 1: 1.0 / 64, 2: 1.0 / 256}
# W-conv integer taps (route PE)
WTAPS = {0: 6.0, 1: 4.0, 2: 1.0}

# ---- tunable schedule config ----
CFG = {
    # per level: {chunk: n PE halves (0..2)}; missing = 0 (all DVE)
    "npe": {0: {1: 2, 4: 2, 6: 1}, 1: {1: 2, 4: 2, 6: 1},
            2: {1: 2, 4: 2, 6: 1}},
    # (level, chunk) -> engine for the first binomial add: "dve" | "pool"
    "a1": {},
    "warmup": 8,
    "tailsplit": 2,
    "split_psum": True,
    "subpool": (),
    "yhp": 4,
    "wtm": 6,
}


def _reflect(i, n):
    if i < 0:
        return -i
    if i >= n:
        return 2 * (n - 1) - i
    return i


def _build_h_bands():
    out = []
    for j, d in enumerate(DILS):
        full = np.zeros((H, H), np.float64)
        for r in range(H):
            for o in (-2 * d, -d, 0, d, 2 * d):
                full[_reflect(r + o, H), r] += HTAPS[abs(o) // d]
        blocks = {}
        for co in range(NCH):
            for ci in range(NCH):
                blk = full[ci * P:(ci + 1) * P, co * P:(co + 1) * P]
                if np.any(blk != 0):
                    blocks[(co, ci)] = np.ascontiguousarray(
                        blk.astype(np.float32))
        out.append(blocks)
    return out


def _pack_consts(h_bands):
    """Pack level-0 bands + identities first so a small head DMA can
    unblock the first chunks; returns (packed, index, wid_off, n0)."""
    mats, seen = [], {}
    index = []
    wid_off = {}

    def add_level(blocks):
        idx = {}
        for key in sorted(blocks):
            b = blocks[key]
            hsh = b.tobytes()
            if hsh not in seen:
                seen[hsh] = len(mats) * P
                mats.append(b)
            idx[key] = seen[hsh]
        return idx

    index.append(add_level(h_bands[0]))
    for t, s in WTAPS.items():
        wid_off[t] = len(mats) * P
        mats.append(np.eye(P, dtype=np.float32) * s)
    n0 = len(mats) * P
    for blocks in h_bands[1:]:
        index.append(add_level(blocks))
    packed = np.ascontiguousarray(
        np.concatenate(mats, axis=1).astype(np.float32))
    return packed, index, wid_off, n0


def _build_program(cfg=None):
    cfg = cfg or CFG
    h_bands = _build_h_bands()
    consts_np, cindex, wid_off, ncols0 = _pack_consts(h_bands)
    ncols_const = consts_np.shape[1]

    nc = bacc.Bacc("TRN2", target_bir_lowering=False, debug=False)
    x_d = nc.dram_tensor("x", [H, W], F32, kind="ExternalInput")
    c_d = nc.dram_tensor("consts", [P, ncols_const], F32,
                         kind="ExternalInput")
    out_d = nc.dram_tensor("out", [LEVELS + 1, H, W], F32,
                           kind="ExternalOutput")

    splitps = cfg.get("split_psum", False)
    with tile.TileContext(nc) as tc:
        with tc.tile_pool(name="sb", bufs=1) as sb, \
             tc.tile_pool(name="yhp", bufs=cfg.get("yhp", 4)) as yhp, \
             tc.tile_pool(name="wtm", bufs=cfg.get("wtm", 6)) as wtm, \
             tc.tile_pool(name="wst", bufs=3) as wstage, \
             tc.tile_pool(name="ps", bufs=(3 if splitps else 4),
                          space="PSUM") as ps, \
             tc.tile_pool(name="psw", bufs=1, space="PSUM") as psw:

            cr = sb.tile([P, ncols_const], BF16, tag="cr", name="cr")
            # level-0 consts first (small, unblocks the first chunks)
            nc.gpsimd.dma_start(cr[:, 0:ncols0], c_d[:, 0:ncols0])

            # ---- PE warm-up: dummy matmuls ramp the clock while the
            # input DMAs are in flight ----
            if cfg.get("warmup"):
                wu = sb.tile([P, 512], BF16, tag="wu", name="wu")
                nc.vector.memset(wu[:], 0)
                pwu = ps.tile([P, 512], F32, tag="psum", name="pwu",
                              bufs=(3 if splitps else 4))
                for _ in range(cfg["warmup"]):
                    nc.tensor.matmul(pwu[:], wu[:, 0:P], wu[:],
                                     start=True, stop=True)

            xr = sb.tile([P, NCH, W], BF16, tag="xr", name="xr")
            cnr = [sb.tile([P, NCH, W], BF16, tag=f"cnr{i}", name=f"cnr{i}")
                   for i in range(2)]

            # ---- load x: 4 quarter cast-DMAs (f32 HBM -> bf16 SBUF),
            # remaining consts after the first two quarters ----
            for q in range(4):
                nc.gpsimd.dma_start(
                    xr[:, 2 * q:2 * q + 2, :],
                    bass.AP(x_d, q * 2 * P * W,
                            [[W, P], [P * W, 2], [1, W]]))
                if q == 1:
                    nc.gpsimd.dma_start(cr[:, ncols0:],
                                        c_d[:, ncols0:])

            for j in range(LEVELS):
                d = DILS[j]
                prev = xr if j == 0 else cnr[(j - 1) % 2]
                cur = cnr[j % 2]
                npe_map = cfg["npe"].get(j, {})

                for c in range(NCH):
                    # ---- H-conv: banded matmuls on PE ----
                    pairs = sorted((key, off)
                                   for key, off in cindex[j].items()
                                   if key[0] == c)
                    ph = ps.tile([P, W], F32, tag="psum", name="ph",
                                 bufs=(3 if splitps else 4))
                    for g in range(2):
                        for i, ((_, ci), off) in enumerate(pairs):
                            nc.tensor.matmul(
                                ph[:, g * 512:(g + 1) * 512],
                                cr[:, off:off + P],
                                prev[:, ci, g * 512:(g + 1) * 512],
                                start=(i == 0),
                                stop=(i == len(pairs) - 1))

                    # ---- evac H into margined tile + reflect margins ----
                    yh = yhp.tile([P, WE], BF16, tag="yh", name="yh")
                    nc.scalar.copy(yh[:, MARG:MARG + W], ph[:])
                    nc.gpsimd.tensor_copy(
                        bass.AP(yh.tensor, 0, [[WE, P], [1, MARG]]),
                        bass.AP(yh.tensor, 2 * MARG, [[WE, P], [-1, MARG]]))
                    nc.gpsimd.tensor_copy(
                        bass.AP(yh.tensor, MARG + W, [[WE, P], [1, MARG]]),
                        bass.AP(yh.tensor, MARG + W - 2,
                                [[WE, P], [-1, MARG]]))

                    def yap(off, ln, yh=yh):
                        return bass.AP(yh.tensor, MARG + off, [[WE, P],
                                                               [1, ln]])

                    npe = npe_map.get(c, 0)
                    if npe:
                        # ---- W-conv route PE: 5 shift-matmuls/half ----
                        if splitps:
                            pw = psw.tile([P, npe * 512], F32, tag="psw",
                                          name="pw", bufs=1)
                        else:
                            pw = ps.tile([P, npe * 512], F32, tag="psum",
                                         name="pw", bufs=4)
                        offs = ((0, 0), (1, -d), (1, d), (2, -2 * d),
                                (2, 2 * d))
                        for g in range(npe):
                            for i, (t, o) in enumerate(offs):
                                nc.tensor.matmul(
                                    pw[:, g * 512:(g + 1) * 512],
                                    cr[:, wid_off[t]:wid_off[t] + P],
                                    yap(g * 512 + o, 512),
                                    start=(i == 0),
                                    stop=(i == len(offs) - 1))
                        nc.scalar.copy(cur[:, c, 0:npe * 512],
                                       pw[:, 0:npe * 512])

                    if npe < 2:
                        # ---- W-conv route DVE: [1,4,6,4,1] = [1,1]^4,
                        # four shifted adds ----
                        a1_pool = cfg["a1"].get((j, c)) == "pool"
                        base = npe * 512
                        wlen = W - base
                        u1 = wtm.tile([P, WE], BF16, tag="u1", name="u1")
                        u2 = wtm.tile([P, WE], BF16, tag="u2", name="u2")

                        def uap(t_, off, ln):
                            return bass.AP(t_.tensor, MARG + off,
                                           [[WE, P], [1, ln]])

                        # u1[t] = y[t] + y[t+d]      t in [base-2d, +wlen+d)
                        eng = nc.gpsimd if a1_pool else nc.vector
                        eng.tensor_add(uap(u1, base - 2 * d, wlen + 3 * d),
                                       yap(base - 2 * d, wlen + 3 * d),
                                       yap(base - d, wlen + 3 * d))
                        # u2[t] = u1[t-d] + u1[t]    t in [base-d, +wlen+d)
                        nc.vector.tensor_add(
                            uap(u2, base - d, wlen + 2 * d),
                            uap(u1, base - 2 * d, wlen + 2 * d),
                            uap(u1, base - d, wlen + 2 * d))
                        # u1[t] = u2[t] + u2[t+d]    t in [base-d, +wlen)
                        nc.vector.tensor_add(
                            uap(u1, base - d, wlen + d),
                            uap(u2, base - d, wlen + d),
                            uap(u2, base, wlen + d))
                        # cur[t] = u1[t-d] + u1[t]   t in [base, +wlen)
                        nc.vector.tensor_add(cur[:, c, base:W],
                                             uap(u1, base - d, wlen),
                                             uap(u1, base, wlen))

                    # ---- subtract + cast DMA out ----
                    last = j == LEVELS - 1
                    # fine-set start must align to pair boundary so no
                    # even chunk is left without a DMA path
                    ts = cfg.get("tailsplit", 0)
                    fine = last and ts and c >= (NCH - ts) // 2 * 2
                    if fine:
                        # chunk-granular tail: shorter final DMA drain
                        wp = wstage.tile([P, 2, W], BF16, tag="wp",
                                         name="wp")
                        nc.vector.tensor_sub(wp[:, 0, :], prev[:, c, :],
                                             cur[:, c, :])
                        nc.gpsimd.dma_start(
                            bass.AP(out_d, 3 * H * W + c * P * W,
                                    [[W, P], [1, W]]),
                            cur[:, c, :])
                        nc.gpsimd.dma_start(
                            bass.AP(out_d, j * H * W + c * P * W,
                                    [[W, P], [1, W]]),
                            wp[:, 0, :])
                    elif c % 2 == 1:
                        c0 = c - 1
                        wp = wstage.tile([P, 2, W], BF16, tag="wp",
                                         name="wp")
                        sub_eng = (nc.gpsimd
                                   if (j, c0 // 2) in cfg.get("subpool", ())
                                   else nc.vector)
                        sub_eng.tensor_sub(wp[:], prev[:, c0:c0 + 2, :],
                                           cur[:, c0:c0 + 2, :])
                        if last:
                            nc.gpsimd.dma_start(
                                bass.AP(out_d, 3 * H * W + c0 * P * W,
                                        [[W, P], [P * W, 2], [1, W]]),
                                cur[:, c0:c0 + 2, :])
                        nc.gpsimd.dma_start(
                            bass.AP(out_d, j * H * W + c0 * P * W,
                                    [[W, P], [P * W, 2], [1, W]]),
                            wp[:])

    nc.compile()
    return nc, consts_np


_CACHE = {}


def _get_program():
    if "prog" not in _CACHE:
        _CACHE["prog"] = _build_program()
    return _CACHE["prog"]


def kernel(x, _trace=False, _trace_kwargs=None):
    """x: [8, 1024, 1024] float32 -> [8, 4, 1024, 1024] float32."""
    x = np.asarray(x)
    assert x.shape == (B, H, W) and x.dtype == np.float32
    nc, consts_np = _get_program()
    in_maps = [{"x": np.ascontiguousarray(x[b]), "consts": consts_np}
               for b in range(B)]
    kw = {}
    if _trace:
        kw = dict(trace=True, **(_trace_kwargs or {}))
    res = run_bass_kernel_spmd(nc, in_maps, core_ids=list(range(B)), **kw)
    out = np.stack([r["out"] for r in res.results], axis=0)
    if _trace:
        return out, res
    return out


# revision 21
# speedup vs baseline: 1.6689x; 1.0003x over previous
"""Self-contained Trainium2 kernel for nn_B3SplineUWT (3-level B3-spline
undecimated wavelet transform), data-parallel over 8 NeuronCores.

kernel(x: [8,1024,1024] f32) -> [8,4,1024,1024] f32  (w1,w2,w3,c3)

Per core: one image, bf16 datapath (tolerance 2e-2; bf16 lands ~2e-3).
  - H-conv (partition dim): PE banded matmuls; taps pre-scaled by 1/16
    (exact in bf16) so the W-conv uses integer taps {1,2,1}x{1,2,1} /
    {1,4,6} with no normalization pass.
  - W-conv (free dim), per 512-half routable:
      route PE : 5 accumulating shift-matmuls (integer-scaled identity
                 stationaries, shifted moving APs)
      route DVE: factorized [1,2,1] twice = 4 adds + 2 scale-by-2
                 (TS ops optionally on ACT, first add optionally on Pool)
  - Subtracts w_j = c_{j-1} - c_j fused per 2-chunk pair on DVE.
  - All HBM I/O via SWDGE cast-DMAs (f32<->bf16 in the DMA), quarter/
    pair granularity for pipelining.
"""
import numpy as np

import concourse.bacc as bacc
import concourse.bass as bass
import concourse.mybir as mybir
import concourse.tile as tile
from concourse.bass_utils import run_bass_kernel_spmd

F32 = mybir.dt.float32
BF16 = mybir.dt.bfloat16
ADD = mybir.AluOpType.add
MULT = mybir.AluOpType.mult

B = 8
H = 1024
W = 1024
P = 128
NCH = H // P
LEVELS = 3
DILS = (1, 2, 4)
MARG = 8
WE = W + 2 * MARG

# H-conv taps with the 1/16 W-normalization folded in (exact in bf16)
HTAPS = {0: 3.0 / 128, 1: 1.0 / 64, 2: 1.0 / 256}
# W-conv integer taps (route PE)
WTAPS = {0: 6.0, 1: 4.0, 2: 1.0}

# ---- tunable schedule config ----
CFG = {
    # per level: {chunk: n PE halves (0..2)}; missing = 0 (all DVE)
    "npe": {0: {1: 2, 4: 2, 6: 1}, 1: {1: 2, 4: 2, 6: 1},
            2: {1: 2, 4: 2, 6: 1}},
    # (level, chunk) -> engine for the first binomial add: "dve" | "pool"
    "a1": {},
    "warmup": 8,
    "tailsplit": 2,
    "split_psum": True,
    "subpool": (),
    "yhp": 4,
    "wtm": 6,
}


def _reflect(i, n):
    if i < 0:
        return -i
    if i >= n:
        return 2 * (n - 1) - i
    return i


def _build_h_bands():
    out = []
    for j, d in enumerate(DILS):
        full = np.zeros((H, H), np.float64)
        for r in range(H):
            for o in (-2 * d, -d, 0, d, 2 * d):
                full[_reflect(r + o, H), r] += HTAPS[abs(o) // d]
        blocks = {}
        for co in range(NCH):
            for ci in range(NCH):
                blk = full[ci * P:(ci + 1) * P, co * P:(co + 1) * P]
                if np.any(blk != 0):
                    blocks[(co, ci)] = np.ascontiguousarray(
                        blk.astype(np.float32))
        out.append(blocks)
    return out


def _pack_consts(h_bands):
    """Pack level-0 bands + identities first so a small head DMA can
    unblock the first chunks; returns (packed, index, wid_off, n0)."""
    mats, seen = [], {}
    index = []
    wid_off = {}

    def add_level(blocks):
        idx = {}
        for key in sorted(blocks):
            b = blocks[key]
            hsh = b.tobytes()
            if hsh not in seen:
                seen[hsh] = len(mats) * P
                mats.append(b)
            idx[key] = seen[hsh]
        return idx

    index.append(add_level(h_bands[0]))
    for t, s in WTAPS.items():
        wid_off[t] = len(mats) * P
        mats.append(np.eye(P, dtype=np.float32) * s)
    n0 = len(mats) * P
    for blocks in h_bands[1:]:
        index.append(add_level(blocks))
    packed = np.ascontiguousarray(
        np.concatenate(mats, axis=1).astype(np.float32))
    return packed, index, wid_off, n0


def _build_program(cfg=None):
    cfg = cfg or CFG
    h_bands = _build_h_bands()
    consts_np, cindex, wid_off, ncols0 = _pack_consts(h_bands)
    ncols_const = consts_np.shape[1]

    nc = bacc.Bacc("TRN2", target_bir_lowering=False, debug=False)
    x_d = nc.dram_tensor("x", [H, W], F32, kind="ExternalInput")
    c_d = nc.dram_tensor("consts", [P, ncols_const], F32,
                         kind="ExternalInput")
    out_d = nc.dram_tensor("out", [LEVELS + 1, H, W], F32,
                           kind="ExternalOutput")

    splitps = cfg.get("split_psum", False)
    with tile.TileContext(nc) as tc:
        with tc.tile_pool(name="sb", bufs=1) as sb, \
             tc.tile_pool(name="yhp", bufs=cfg.get("yhp", 4)) as yhp, \
             tc.tile_pool(name="wtm", bufs=cfg.get("wtm", 6)) as wtm, \
             tc.tile_pool(name="wst", bufs=3) as wstage, \
             tc.tile_pool(name="ps", bufs=(3 if splitps else 4),
                          space="PSUM") as ps, \
             tc.tile_pool(name="psw", bufs=1, space="PSUM") as psw:

            cr = sb.tile([P, ncols_const], BF16, tag="cr", name="cr")
            # level-0 consts first (small, unblocks the first chunks)
            nc.gpsimd.dma_start(cr[:, 0:ncols0], c_d[:, 0:ncols0])

            # ---- PE warm-up: dummy matmuls ramp the clock while the
            # input DMAs are in flight ----
            if cfg.get("warmup"):
                wu = sb.tile([P, 512], BF16, tag="wu", name="wu")
                nc.vector.memset(wu[:], 0)
                pwu = ps.tile([P, 512], F32, tag="psum", name="pwu",
                              bufs=(3 if splitps else 4))
                for _ in range(cfg["warmup"]):
                    nc.tensor.matmul(pwu[:], wu[:, 0:P], wu[:],
                                     start=True, stop=True)

            xr = sb.tile([P, NCH, W], BF16, tag="xr", name="xr")
            cnr = [sb.tile([P, NCH, W], BF16, tag=f"cnr{i}", name=f"cnr{i}")
                   for i in range(2)]

            # ---- load x: 4 quarter cast-DMAs (f32 HBM -> bf16 SBUF),
            # remaining consts after all quarters (needed from level 1) ----
            for q in range(4):
                nc.gpsimd.dma_start(
                    xr[:, 2 * q:2 * q + 2, :],
                    bass.AP(x_d, q * 2 * P * W,
                            [[W, P], [P * W, 2], [1, W]]))
            nc.gpsimd.dma_start(cr[:, ncols0:], c_d[:, ncols0:])

            for j in range(LEVELS):
                d = DILS[j]
                prev = xr if j == 0 else cnr[(j - 1) % 2]
                cur = cnr[j % 2]
                npe_map = cfg["npe"].get(j, {})

                for c in range(NCH):
                    # ---- H-conv: banded matmuls on PE ----
                    pairs = sorted((key, off)
                                   for key, off in cindex[j].items()
                                   if key[0] == c)
                    ph = ps.tile([P, W], F32, tag="psum", name="ph",
                                 bufs=(3 if splitps else 4))
                    for g in range(2):
                        for i, ((_, ci), off) in enumerate(pairs):
                            nc.tensor.matmul(
                                ph[:, g * 512:(g + 1) * 512],
                                cr[:, off:off + P],
                                prev[:, ci, g * 512:(g + 1) * 512],
                                start=(i == 0),
                                stop=(i == len(pairs) - 1))

                    # ---- evac H into margined tile + reflect margins ----
                    yh = yhp.tile([P, WE], BF16, tag="yh", name="yh")
                    nc.scalar.copy(yh[:, MARG:MARG + W], ph[:])
                    meng = (nc.scalar if cfg.get("marg") == "act"
                            else nc.gpsimd)
                    mcopy = (meng.copy if meng is nc.scalar
                             else meng.tensor_copy)
                    mcopy(
                        bass.AP(yh.tensor, 0, [[WE, P], [1, MARG]]),
                        bass.AP(yh.tensor, 2 * MARG, [[WE, P], [-1, MARG]]))
                    mcopy(
                        bass.AP(yh.tensor, MARG + W, [[WE, P], [1, MARG]]),
                        bass.AP(yh.tensor, MARG + W - 2,
                                [[WE, P], [-1, MARG]]))

                    def yap(off, ln, yh=yh):
                        return bass.AP(yh.tensor, MARG + off, [[WE, P],
                                                               [1, ln]])

                    npe = npe_map.get(c, 0)
                    if npe:
                        # ---- W-conv route PE: 5 shift-matmuls/half ----
                        if splitps:
                            pw = psw.tile([P, npe * 512], F32, tag="psw",
                                          name="pw", bufs=1)
                        else:
                            pw = ps.tile([P, npe * 512], F32, tag="psum",
                                         name="pw", bufs=4)
                        offs = ((0, 0), (1, -d), (1, d), (2, -2 * d),
                                (2, 2 * d))
                        for g in range(npe):
                            for i, (t, o) in enumerate(offs):
                                nc.tensor.matmul(
                                    pw[:, g * 512:(g + 1) * 512],
                                    cr[:, wid_off[t]:wid_off[t] + P],
                                    yap(g * 512 + o, 512),
                                    start=(i == 0),
                                    stop=(i == len(offs) - 1))
                        nc.scalar.copy(cur[:, c, 0:npe * 512],
                                       pw[:, 0:npe * 512])

                    if npe < 2:
                        # ---- W-conv route DVE: [1,4,6,4,1] = [1,1]^4,
                        # four shifted adds ----
                        a1_pool = cfg["a1"].get((j, c)) == "pool"
                        base = npe * 512
                        wlen = W - base
                        u1 = wtm.tile([P, WE], BF16, tag="u1", name="u1")
                        u2 = wtm.tile([P, WE], BF16, tag="u2", name="u2")

                        def uap(t_, off, ln):
                            return bass.AP(t_.tensor, MARG + off,
                                           [[WE, P], [1, ln]])

                        # u1[t] = y[t] + y[t+d]      t in [base-2d, +wlen+d)
                        eng = nc.gpsimd if a1_pool else nc.vector
                        eng.tensor_add(uap(u1, base - 2 * d, wlen + 3 * d),
                                       yap(base - 2 * d, wlen + 3 * d),
                                       yap(base - d, wlen + 3 * d))
                        # u2[t] = u1[t-d] + u1[t]    t in [base-d, +wlen+d)
                        nc.vector.tensor_add(
                            uap(u2, base - d, wlen + 2 * d),
                            uap(u1, base - 2 * d, wlen + 2 * d),
                            uap(u1, base - d, wlen + 2 * d))
                        # u1[t] = u2[t] + u2[t+d]    t in [base-d, +wlen)
                        nc.vector.tensor_add(
                            uap(u1, base - d, wlen + d),
                            uap(u2, base - d, wlen + d),
                            uap(u2, base, wlen + d))
                        # cur[t] = u1[t-d] + u1[t]   t in [base, +wlen)
                        nc.vector.tensor_add(cur[:, c, base:W],
                                             uap(u1, base - d, wlen),
                                             uap(u1, base, wlen))

                    # ---- subtract + cast DMA out ----
                    last = j == LEVELS - 1
                    # fine-set start must align to pair boundary so no
                    # even chunk is left without a DMA path
                    ts = cfg.get("tailsplit", 0)
                    fine = last and ts and c >= (NCH - ts) // 2 * 2
                    if fine:
                        # chunk-granular tail: shorter final DMA drain
                        wp = wstage.tile([P, 2, W], BF16, tag="wp",
                                         name="wp")
                        nc.vector.tensor_sub(wp[:, 0, :], prev[:, c, :],
                                             cur[:, c, :])
                        nc.gpsimd.dma_start(
                            bass.AP(out_d, 3 * H * W + c * P * W,
                                    [[W, P], [1, W]]),
                            cur[:, c, :])
                        nc.gpsimd.dma_start(
                            bass.AP(out_d, j * H * W + c * P * W,
                                    [[W, P], [1, W]]),
                            wp[:, 0, :])
                    elif c % 2 == 1:
                        c0 = c - 1
                        wp = wstage.tile([P, 2, W], BF16, tag="wp",
                                         name="wp")
                        sub_eng = (nc.gpsimd
                                   if (j, c0 // 2) in cfg.get("subpool", ())
                                   else nc.vector)
                        sub_eng.tensor_sub(wp[:], prev[:, c0:c0 + 2, :],
                                           cur[:, c0:c0 + 2, :])
                        if last:
                            nc.gpsimd.dma_start(
                                bass.AP(out_d, 3 * H * W + c0 * P * W,
                                        [[W, P], [P * W, 2], [1, W]]),
                                cur[:, c0:c0 + 2, :])
                        nc.gpsimd.dma_start(
                            bass.AP(out_d, j * H * W + c0 * P * W,
                                    [[W, P], [P * W, 2], [1, W]]),
                            wp[:])

    nc.compile()
    return nc, consts_np


_CACHE = {}


def _get_program():
    if "prog" not in _CACHE:
        _CACHE["prog"] = _build_program()
    return _CACHE["prog"]


def kernel(x, _trace=False, _trace_kwargs=None):
    """x: [8, 1024, 1024] float32 -> [8, 4, 1024, 1024] float32."""
    x = np.asarray(x)
    assert x.shape == (B, H, W) and x.dtype == np.float32
    nc, consts_np = _get_program()
    in_maps = [{"x": np.ascontiguousarray(x[b]), "consts": consts_np}
               for b in range(B)]
    kw = {}
    if _trace:
        kw = dict(trace=True, **(_trace_kwargs or {}))
    res = run_bass_kernel_spmd(nc, in_maps, core_ids=list(range(B)), **kw)
    out = np.stack([r["out"] for r in res.results], axis=0)
    if _trace:
        return out, res
    return out
